# revision 35
# baseline (speedup 1.0000x reference)
"""FlowNetC correlation layer on 8 Trainium2 NeuronCores.

Math: out[b, d, y, x] = (1/256) * sum_c in1[b,c,y,x] * in2pad[b,c,y+dy,x+dx]
with (dy, dx) on a 21x21 stride-2 grid spanning [-20, 20], zero padding 20.

Strategy (per core = one batch sample; batch is exactly 8):
- Displacements have stride 2, so the problem splits into 4 independent parity
  classes. Each class: in1c [256, 32, 48] against in2c [256, 32, 68] (x-padded
  only, zero-padded 10 per side; y-pad rows are never multiplied -- see below)
  with stride-1 displacements dy', dx' in [0, 20].
- Gram band matmuls: per class and group of 4 subsampled x-columns, 4
  col-tiled matmuls (M=32 each, tile_position=(0, 32*xg)); on HW the 4
  column tiles stream concurrently (measured: mm-only marginal ~= N cycles
  / 4). Stationary is in1c[:, :, x0] (32 ys); moving is the 21-wide window
  in2c[:, :, x0:x0+21] over the 32 VALID rows only (N = 672 = 441 + 231
  across 2 PSUM banks). Zero-pad rows would only produce zeros, so they are
  never streamed; the zeros are materialized once in the HBM dump instead.
- Evictions psum->band alternate DVE/ACT; band holds only the valid rows
  (j = r*21+dx, r in [0,32)).
- De-shear via a shear-absorbing HBM bounce: the valid band regions are
  written per class (in two group-halves, the first mid-MM-phase) into a
  52-row dump laid out [p][xsg][R*21+dx] (strided dst, R in [10,42)); the
  y-pad regions R in [0,10) u [42,52) are zeroed ONCE per dump slot at
  kernel start. 8 read-backs (per xg and group-half, each gated only on
  its own half's write) with HBM-side partition stride FB+21 bake the
  21*ys shear into the addressing (SBUF partition steps must be pitch
  multiples, HBM strides are unconstrained); each pixel's 441-displacement
  window is contiguous in the dump.
- TensorE transposes flip dense [pixel, d] tiles to [d, pixel], deferred
  PEND_DEPTH classes so the PE's in-order stream has MM-phase slack over
  the bounce chain; one batched scatter copy per d-chunk (DVE/ACT
  alternating, 12 transposed groups at a time) assembles a d-major bf16
  raster; 4 output DMAs write bf16 [441, 64, 96] (cast to f32 on host)
  with 12 KB contiguous runs per d.
- in1 AND in2 are fully SBUF-resident (loaded once, not per class/rep).
- Matmul inputs are bf16; the 1/256 normalization is folded into in1's bf16
  cast exactly (exponent shift).
"""

import os
import sys

for _p in ("/opt/trn_rl_repo", "/root/.axon_site/_ro/trn_rl_repo"):
    if os.path.isdir(_p) and _p not in sys.path:
        sys.path.insert(0, _p)

from contextlib import ExitStack

import ml_dtypes
import numpy as np

import concourse.bacc as bacc
import concourse.bass as bass
import concourse.mybir as mybir
import concourse.tile as tile
from concourse.bass_utils import run_bass_kernel_spmd
from concourse.masks import make_identity

B, C, H, W = 8, 256, 64, 96
NYS, NXS = 32, 48          # subsampled class grid
RB, CB = 52, 68            # padded class grid (rows/cols) in the dump / x-pad
NVR = 32                   # valid rows per class (y-pad rows excluded)
ND = 441                   # displacements
WB = RB * 21               # dump band width per xs-column (52 rows * 21 dx)
VWB = NVR * 21             # valid band width per xs-column (672)
NG = 12                    # xs-column groups per class band
FB = NG * WB               # dump pitch per partition (13104)
BANDW = NG * VWB           # SBUF band free size (8064)
DP = NG * ND               # dense free size (5292)
NPIX = H * W               # 6144
DCHUNKS = [(0, 128), (128, 128), (256, 128), (384, 57)]
GRAM_CHUNKS = [(0, 21), (21, 32)]  # valid-row ranges per PSUM bank
ZPAD0, ZPAD1 = 10 * 21, RB * 21 - 10 * 21  # zero region sizes: [0,210), [882,1092)

F32 = mybir.dt.float32
BF16 = mybir.dt.bfloat16
PEND_DEPTH = 2  # classes of transpose/scatter deferral over the bounce chain


def build(reps=1, mm_only=False, stage=3):
    """stage: 0 = matmul+evict only, 1 = +bounce write/read-back,
    2 = +transpose/scatter, 3 = full (output DMAs). mm_only is stage 0."""
    if mm_only:
        stage = 0
    nc = bacc.Bacc("TRN2", target_bir_lowering=False, debug=False, num_devices=8)
    in1p = nc.declare_dram_parameter("in1p", [2, 128, 4, NXS, NYS], BF16, isOutput=False)
    in2p = nc.declare_dram_parameter("in2p", [128, 4, 2, NVR, CB], BF16, isOutput=False)
    outp = nc.declare_dram_parameter("out", [ND, H, W], BF16, isOutput=True)

    with tile.TileContext(nc) as tc:
        with ExitStack() as ctx:
            const_pool = ctx.enter_context(tc.tile_pool(name="const", bufs=1))
            hbm_pool = ctx.enter_context(tc.tile_pool(name="hbm", bufs=2, space="DRAM"))
            pg_pool = ctx.enter_context(tc.tile_pool(name="pg", bufs=2, space="PSUM"))
            pt_pool = ctx.enter_context(tc.tile_pool(name="pt", bufs=2, space="PSUM"))

            ident = const_pool.tile([128, 128], BF16)
            make_identity(nc, ident)

            # resident in1: [c, k, cls, xs, ys]
            in1_sb = const_pool.tile([128, 2, 4, NXS, NYS], BF16)
            nc.sync.dma_start(
                out=bass.AP(in1_sb.tensor, in1_sb.offset,
                            [[2 * 4 * NXS * NYS, 128], [4 * NXS * NYS, 2], [1, 4 * NXS * NYS]]),
                in_=bass.AP(in1p, 0,
                            [[4 * NXS * NYS, 128], [128 * 4 * NXS * NYS, 2], [1, 4 * NXS * NYS]]),
            )
            # resident in2: [c, cls, k, r, col] (valid rows, x-padded)
            IN2F = 4 * 2 * NVR * CB
            in2_sb = const_pool.tile([128, 4, 2, NVR, CB], BF16)
            nc.scalar.dma_start(
                out=bass.AP(in2_sb.tensor, in2_sb.offset, [[IN2F, 128], [1, IN2F]]),
                in_=bass.AP(in2p, 0, [[IN2F, 128], [1, IN2F]]),
            )

            # zero source for the one-time dump pad init
            zero_sb = const_pool.tile([128, NG * ZPAD0], BF16)
            nc.gpsimd.memset(zero_sb, 0.0)

            # persistent d-major assembly buffers, one per d-chunk
            out_sb = [const_pool.tile([128, NPIX], BF16, name=f"out_sb{dc}")
                      for dc in range(4)]

            # persistent double-buffered band/dense (the de-shear DMAs use
            # partition-strided raw APs the tile tracker can't attribute, so
            # pooled slot-reuse is unsafe; ordering is via explicit deps)
            bands = [const_pool.tile([128, BANDW], BF16, name=f"band{i}")
                     for i in range(2)]
            denses = [const_pool.tile([128, NG, ND], BF16, name=f"dense{i}")
                      for i in range(4)]
            hbs = [hbm_pool.tile([128, FB], BF16, name=f"hb{i}")
                   for i in range(2)]

            # one-time: zero the dump's y-pad regions (R in [0,10) u [42,52))
            zinit = [[], []]
            for hs in range(2):
                hb = hbs[hs]
                for off in (0, ZPAD1):
                    z = nc.gpsimd.dma_start(
                        out=bass.AP(hb.tensor, hb.offset + off,
                                    [[FB, 128], [WB, NG], [1, ZPAD0]]),
                        in_=bass.AP(zero_sb.tensor, zero_sb.offset,
                                    [[NG * ZPAD0, 128], [1, NG * ZPAD0]]),
                    )
                    zinit[hs].append(z)

            slot_wr = [[], []]      # bounce writes per band slot (WAR for evictions)
            slot_rb = [[], []]      # read-backs per hb slot (WAR for writes)
            slot_tr = [[], [], [], []]  # transposes per dense slot (WAR for read-backs)

            def transpose_scatter(cid, dense, dss_halves, eng_flip, emit_out=False):
                """PE transposes dense [pixel, d] to [d, pixel]; DVE/ACT
                scatter into the d-major raster assembly buffers. With
                emit_out (final class of a rep), each d-chunk's output DMA
                is issued right after its scatter instead of after all of
                them, trimming the rep-end tail."""
                py, px = cid // 2, cid % 2
                trs = []
                for dc, (d0, dcw) in enumerate(DCHUNKS):
                    pt = pt_pool.tile([128, NG * 128], BF16)
                    for g in range(NG):
                        tr = nc.tensor.transpose(
                            pt[0:dcw, g * 128:(g + 1) * 128],
                            dense[:, g, d0:d0 + dcw],
                            ident[:],
                        )
                        for ds in dss_halves[g // 6]:
                            tile.add_dep_helper(tr.ins, ds.ins,
                                                reason="transpose needs de-shear")
                        trs.append(tr.ins)
                    ob = out_sb[dc]
                    src = bass.AP(pt.tensor, pt.offset,
                                  [[NG * 128, dcw], [128, NG], [32, 4], [1, 32]])
                    doff = 96 * py + px
                    dst = bass.AP(ob.tensor, ob.offset + doff,
                                  [[NPIX, dcw], [8, NG], [2, 4], [192, 32]])
                    if eng_flip % 2 == 0:
                        nc.vector.tensor_copy(out=dst, in_=src)
                    else:
                        nc.scalar.copy(out=dst, in_=src)
                    eng_flip += 1
                    if emit_out:
                        d0 = DCHUNKS[dc][0]
                        nc.gpsimd.dma_start(
                            out=bass.AP(outp, d0 * NPIX, [[NPIX, dcw], [1, NPIX]]),
                            in_=bass.AP(ob.tensor, ob.offset, [[NPIX, dcw], [1, NPIX]]),
                        )
                slot_tr[cid % 4] = trs
                return eng_flip

            eng_flip = 0
            pend = []  # deferred (cid, dense, dss) transpose stages, depth 2
            for rep in range(reps):
              for cid in range(4):
                slot = cid % 2
                band = bands[slot]
                hb = hbs[slot]
                evs = []
                wrs = []
                for xsg in range(12):
                    pg = pg_pool.tile([128, 2, 512], F32)
                    # xg innermost: consecutive matmuls target different PE
                    # column tiles, so their moving streams overlap in the
                    # array (same-tile chunks would serialize).
                    for k in range(2):
                        for ch, (r0, r1) in enumerate(GRAM_CHUNKS):
                            ncols = (r1 - r0) * 21
                            for xg in range(4):
                                x0 = 4 * xsg + xg
                                lhsT = in1_sb[:, k, cid, x0, :]
                                rhs = in2_sb[:, cid, k, r0:r1, x0:x0 + 21]
                                nc.tensor.matmul(
                                    pg[32 * xg:32 * (xg + 1), ch, 0:ncols],
                                    lhsT, rhs,
                                    start=(k == 0), stop=(k == 1),
                                    tile_position=(0, 32 * xg),
                                    skip_group_check=True,
                                )
                    # evict psum into packed valid-row band columns; big
                    # chunk (441) and small chunk (231) on opposite engines,
                    # alternating per xsg for balance.
                    big_src = pg[:, 0, 0:441]
                    big_dst = band[:, xsg * VWB: xsg * VWB + 441]
                    small_src = pg[:, 1, 0:231]
                    small_dst = band[:, xsg * VWB + 441: xsg * VWB + 672]
                    if xsg % 2 == 0:
                        evs.append(nc.vector.tensor_copy(out=big_dst, in_=big_src))
                        evs.append(nc.scalar.copy(out=small_dst, in_=small_src))
                    else:
                        evs.append(nc.scalar.copy(out=big_dst, in_=big_src))
                        evs.append(nc.vector.tensor_copy(out=small_dst, in_=small_src))
                    for ev in evs[-2:]:
                        for wr in slot_wr[slot]:
                            tile.add_dep_helper(ev.ins, wr.ins,
                                                reason="eviction WAR on prior bounce write")
                    if stage >= 1 and xsg in (2, 5, 8, 11):
                        # bounce write, in group-quarters: each fires as soon
                        # as its 3 blocks' evictions land, so by the time a
                        # half's read-backs are gated only ~half the half's
                        # transfer remains. dst is strided: valid rows land
                        # at [210, 882) of each 1092-wide dump block.
                        q = xsg // 3
                        wr = nc.sync.dma_start(
                            out=bass.AP(hb.tensor, hb.offset + q * 3 * WB + ZPAD0,
                                        [[FB, 128], [WB, 3], [1, VWB]]),
                            in_=bass.AP(band.tensor, band.offset + q * 3 * VWB,
                                        [[BANDW, 128], [1, 3 * VWB]]),
                        )
                        for ev in evs[6 * q:6 * q + 6]:
                            tile.add_dep_helper(wr.ins, ev.ins,
                                                reason="bounce write needs quarter's evictions")
                        for rb in slot_rb[slot]:
                            tile.add_dep_helper(wr.ins, rb.ins,
                                                reason="bounce write WAR on prior read-backs")
                        for z in zinit[slot]:
                            tile.add_dep_helper(wr.ins, z.ins,
                                                reason="bounce write after pad zero-init")
                        wrs.append(wr)

                if stage == 0:
                    continue
                # de-shear read-backs: 8 DMAs (per xg and group-half) whose
                # HBM-side partition stride FB+21 bakes the 21*ys shear into
                # the addressing (SBUF partition steps must be pitch
                # multiples — neuronxcc's BIR verifier rejects sheared SBUF
                # APs — HBM strides are unconstrained). Each half only needs
                # its own bounce write, so the first 4 read-backs start
                # mid-class.
                dense = denses[cid % 4]
                dss_halves = [[], []]
                for h in range(2):
                    for xg in range(4):
                        src = bass.AP(hb.tensor,
                                      hb.offset + 32 * xg * FB + h * 6 * WB,
                                      [[FB + 21, 32], [WB, 6], [1, ND]])
                        dst = bass.AP(dense.tensor,
                                      dense.offset + 32 * xg * DP + h * 6 * ND,
                                      [[DP, 32], [ND, 6], [1, ND]])
                        eng = nc.scalar if xg % 2 == 0 else nc.sync
                        rd = eng.dma_start(out=dst, in_=src)
                        for wr in wrs[2 * h:2 * h + 2]:
                            tile.add_dep_helper(rd.ins, wr.ins,
                                                reason="read-back needs its half's writes")
                        for z in zinit[slot]:
                            tile.add_dep_helper(rd.ins, z.ins,
                                                reason="read-back after pad zero-init")
                        for tr in slot_tr[cid % 4]:
                            tile.add_dep_helper(rd.ins, tr,
                                                reason="read-back WAR on prior transposes")
                        dss_halves[h].append(rd)
                slot_rb[slot] = dss_halves[0] + dss_halves[1]
                slot_wr[slot] = wrs

                if stage == 1:
                    # timing stub: chain WAR through the read-backs themselves
                    slot_tr[cid % 4] = [rd.ins for rd in slot_rb[slot]]
                    continue
                # transpose/scatter deferred PEND_DEPTH classes so PE's
                # in-order stream has slack over the bounce chain.
                if len(pend) == PEND_DEPTH:
                    eng_flip = transpose_scatter(*pend.pop(0), eng_flip)
                pend.append((cid, dense, dss_halves))

              while pend:
                  args = pend.pop(0)
                  eng_flip = transpose_scatter(
                      *args, eng_flip, emit_out=(not pend and stage == 3))

              if stage == 0:
                  # timing stub: drain one band slice so the NEFF has output
                  nc.gpsimd.dma_start(
                      out=bass.AP(outp, 0, [[NPIX, 128], [1, NPIX]]),
                      in_=bass.AP(band.tensor, band.offset, [[BANDW, 128], [1, NPIX]]),
                  )
                  continue
              if stage == 1:
                  nc.gpsimd.dma_start(
                      out=bass.AP(outp, 0, [[NPIX, 128], [1, 4096]]),
                      in_=bass.AP(denses[0].tensor, denses[0].offset,
                                  [[DP, 128], [1, 4096]]),
                  )
                  continue
              if stage == 2:
                  nc.gpsimd.dma_start(
                      out=bass.AP(outp, 0, [[NPIX, 128], [1, NPIX]]),
                      in_=bass.AP(out_sb[0].tensor, out_sb[0].offset,
                                  [[NPIX, 128], [1, NPIX]]),
                  )
                  continue
              # output DMAs (12KB contiguous runs per d) were emitted inside
              # the final transpose_scatter via emit_out; nothing to do here.

    nc.compile()
    return nc


def prep_inputs(input1, input2):
    """Host-side: parity split, x-pad, bf16 cast, fold 1/256 into in1."""
    in_maps = []
    for b in range(B):
        a1 = (input1[b].astype(np.float32) / 256.0).reshape(2, 128, H, W)
        a2 = input2[b].astype(np.float32).reshape(2, 128, H, W)
        in1p = np.empty((2, 128, 4, NXS, NYS), dtype=ml_dtypes.bfloat16)
        in2p = np.zeros((128, 4, 2, NVR, CB), dtype=ml_dtypes.bfloat16)
        for cid in range(4):
            py, px = cid // 2, cid % 2
            c1 = a1[:, :, py::2, px::2]  # [2, 128, 32, 48]
            c2 = a2[:, :, py::2, px::2]
            in1p[:, :, cid] = c1.transpose(0, 1, 3, 2).astype(ml_dtypes.bfloat16)
            in2p[:, cid, :, :, 10:58] = c2.transpose(1, 0, 2, 3).astype(ml_dtypes.bfloat16)
        in_maps.append({"in1p": in1p, "in2p": in2p})
    return in_maps


_NC = None


def get_nc():
    global _NC
    if _NC is None:
        _NC = build()
    return _NC


def kernel(input1, input2):
    nc = get_nc()
    in_maps = prep_inputs(np.asarray(input1), np.asarray(input2))
    r = run_bass_kernel_spmd(nc, in_maps, core_ids=list(range(8)))
    return np.stack([r.results[i]["out"] for i in range(B)]).astype(np.float32)
